# revision 10
# baseline (speedup 1.0000x reference)
"""Trainium2 Bass kernel for BiochemicalConstraintLayer (GNN message passing).

Computation (see reference.py):
  valences   = softmax(relu(nf @ vw1 + vb1) @ vw2 + vb2)              [N, 8]
  bond_types = softmax(relu(cat(nf[row], nf[col]) @ bw1 + bb1) @ bw2 + bb2)  [E, 4]
  bond_order = bond_types @ [1, 2, 3, 1.5]                            [E]
  node_deg   = scatter_add(bond_order at row)                         [N]
  violation  = mean((node_deg - (argmax(valences)+1))^2)              scalar

Key algebraic trick: cat(nf[row], nf[col]) @ bw1 == A[row] + B[col] where
A = nf @ bw1[:128], B = nf @ bw1[128:].  Edge-parallel over 8 cores; each
core:
  phase A: dense matmuls build the fused per-node table
           tab[n, 0:64] = [A[n]+bb1 | B[n]] in bf16 (256B rows),
  phase B: valence MLP over the core's 1/8 node shard (f32),
  phase C: per-edge gathers of tab rows with the MoE dma_gather
           (int16 indices force <=32768-row tables, so the host pre-splits
           each core's edge shard into 4 groups by (row-half, col-half)),
           then PE transposes + a block-diagonal 32->4 matmul + softmax
           produce bond_types.
Degrees: hardware scatter-add paths lose duplicate-address adds (verified
by probes on trn2: both indirect DMA with CCE add and the Q7
dma_scatter_add drop concurrent same-address updates), so node degrees are
accumulated on the host with np.bincount from the returned bond_types.
"""

import os
import sys
import numpy as np

sys.path.insert(0, "/opt/trn_rl_repo")

import ml_dtypes

from concourse import bass, mybir, bacc, tile
from concourse.masks import make_identity

FP32 = mybir.dt.float32
BF16 = mybir.dt.bfloat16
I32 = mybir.dt.int32
I16 = mybir.dt.int16

BOND_W = (1.0, 2.0, 3.0, 1.5)


def default_params():
    # Full-size problem: N=50000 nodes, E=800000 edges, 8 cores.
    return dict(
        n_cores=8,
        n_nodes=50000,
        e_per_core=100000,
        epg=26624,     # padded edges per (row-half, col-half) group: 128*208
        sw=52,         # edge tokens per partition per slab -> 4 slabs/group
        node_shard=6250,
        nsp=6656,      # padded node shard (13 tiles of 512)
        ntab=50176,    # padded table rows (14 groups of 3584); half = 25088
        ga=3584,       # phase-A group: nodes per load group (28 matmul tiles)
    )


def small_params():
    # Scaled-down config for simulator testing.
    return dict(
        n_cores=1,
        n_nodes=500,
        e_per_core=4000,
        epg=1536,      # 128*12
        sw=4,          # 3 slabs/group
        node_shard=500,
        nsp=512,
        ntab=512,      # half = 256
        ga=256,
    )


def build_nc(P, num_devices=None):
    """Build the SPMD Bass program (same program on every core)."""
    NT = P["ntab"]
    HREF = NT // 2
    EPG = P["epg"]
    SW = P["sw"]
    NSP = P["nsp"]
    GA = P["ga"]
    assert EPG % 128 == 0
    assert (EPG // 128) % SW == 0
    nslab = EPG // 128 // SW       # slabs per group
    TOK = 128 * SW                 # tokens per slab
    EPALL = 4 * EPG                # padded edge tokens per core
    assert NT % GA == 0
    ngroup = NT // GA
    assert GA % 128 == 0
    assert NSP % 512 == 0
    nvt = NSP // 512
    assert SW % 4 == 0
    nzt = SW // 4                  # 128-wide transpose blocks per slab

    nc = bacc.Bacc(
        "TRN2",
        target_bir_lowering=False,
        debug=False,
        enable_asserts=True,
        num_devices=num_devices or P["n_cores"],
    )

    # ---- I/O ----
    nfT = nc.dram_tensor("nfT", [128, NT], BF16, kind="ExternalInput").ap()
    nfsT = nc.dram_tensor("nfsT", [128, NSP], FP32, kind="ExternalInput").ap()
    idxr16 = nc.dram_tensor("idxr16", [128, EPALL // 16], I16,
                            kind="ExternalInput").ap()
    idxc16 = nc.dram_tensor("idxc16", [128, EPALL // 16], I16,
                            kind="ExternalInput").ap()
    w1ab = nc.dram_tensor("w1ab", [128, 64], BF16, kind="ExternalInput").ap()
    biasAB = nc.dram_tensor("biasAB", [128, 64], FP32, kind="ExternalInput").ap()
    vw1 = nc.dram_tensor("vw1", [128, 32], FP32, kind="ExternalInput").ap()
    vb1b = nc.dram_tensor("vb1b", [128, 32], FP32, kind="ExternalInput").ap()
    vw2 = nc.dram_tensor("vw2", [32, 8], FP32, kind="ExternalInput").ap()
    vb2b = nc.dram_tensor("vb2b", [128, 8], FP32, kind="ExternalInput").ap()
    w2blk = nc.dram_tensor("w2blk", [128, 16], BF16, kind="ExternalInput").ap()
    bb2b = nc.dram_tensor("bb2b", [128, 16], FP32, kind="ExternalInput").ap()

    val_out = nc.dram_tensor("val_out", [NSP, 8], FP32, kind="ExternalOutput").ap()
    bt_out = nc.dram_tensor("bt_out", [EPALL, 4], FP32, kind="ExternalOutput").ap()

    tab = nc.dram_tensor("tab", [NT, 128], BF16).ap()

    with tile.TileContext(nc) as tc:
        with (
            tc.tile_pool(name="const", bufs=1) as cpool,
            tc.tile_pool(name="work", bufs=2) as wpool,
            tc.tile_pool(name="psum", bufs=3, space="PSUM") as ppool,
        ):
            # ---- constants in SBUF ----
            ident = cpool.tile([128, 128], BF16, tag="identb")
            make_identity(nc, ident[:])
            identf = cpool.tile([128, 128], FP32, tag="identf")
            make_identity(nc, identf[:])
            w1ab_s = cpool.tile([128, 64], BF16, tag="w1ab")
            nc.sync.dma_start(w1ab_s[:], w1ab)
            biasAB_s = cpool.tile([128, 64], FP32, tag="biasAB")
            nc.sync.dma_start(biasAB_s[:], biasAB)
            vw1_s = cpool.tile([128, 32], FP32, tag="vw1")
            nc.sync.dma_start(vw1_s[:], vw1)
            vb1b_s = cpool.tile([128, 32], FP32, tag="vb1b")
            nc.sync.dma_start(vb1b_s[:], vb1b)
            vw2_s = cpool.tile([32, 8], FP32, tag="vw2")
            nc.sync.dma_start(vw2_s[:], vw2)
            vb2b_s = cpool.tile([128, 8], FP32, tag="vb2b")
            nc.sync.dma_start(vb2b_s[:], vb2b)
            w2blk_s = cpool.tile([128, 16], BF16, tag="w2blk")
            nc.sync.dma_start(w2blk_s[:], w2blk)
            bb2b_s = cpool.tile([128, 16], FP32, tag="bb2b")
            nc.sync.dma_start(bb2b_s[:], bb2b)

            # ---- phase A: tab[n, 0:64] = [nf@w1_top + bb1 | nf@w1_bot] ----
            nsub = GA // 128
            for g in range(ngroup):
                nft_g = wpool.tile([128, GA], BF16, tag="nft_g")
                nc.sync.dma_start(nft_g[:], nfT[:, g * GA:(g + 1) * GA])
                tab_g = wpool.tile([128, nsub * 64], BF16, tag="tab_g")
                for q in range(0, nsub, 7):
                    qn = min(7, nsub - q)
                    psA = ppool.tile([128, qn * 64], FP32, tag="psbig")
                    for b in range(qn):
                        nc.tensor.matmul(
                            psA[:, b * 64:(b + 1) * 64],
                            lhsT=nft_g[:, (q + b) * 128:(q + b + 1) * 128],
                            rhs=w1ab_s[:],
                            start=True,
                            stop=True,
                        )
                    nc.vector.tensor_tensor(
                        out=tab_g[:, q * 64:(q + qn) * 64],
                        in0=psA[:],
                        in1=biasAB_s[:].rearrange("p (j c) -> p j c", j=1)
                        .to_broadcast([128, qn, 64]),
                        op=mybir.AluOpType.add,
                    )
                # store: node n = g*GA + 128*b + p -> tab row n, cols 0:64
                nc.sync.dma_start(
                    tab[g * GA:(g + 1) * GA, 0:64]
                    .rearrange("(b p) c -> p b c", p=128),
                    tab_g[:].rearrange("p (b c) -> p b c", c=64),
                )

            # ---- phase B: valence MLP on this core's node shard ----
            nfsT_s = wpool.tile([128, NSP], FP32, tag="nfsT")
            nc.sync.dma_start(nfsT_s[:], nfsT)
            smxv = wpool.tile([128, nvt * 4 * 8], FP32, tag="smxv")
            for t in range(nvt):
                for b in range(4):
                    col = 512 * t + 128 * b
                    psV = ppool.tile([128, 32], FP32, tag="psmall")
                    nc.tensor.matmul(
                        psV[:],
                        lhsT=nfsT_s[:, col:col + 128],
                        rhs=vw1_s[:],
                        start=True,
                        stop=True,
                    )
                    h1 = wpool.tile([128, 32], FP32, tag="h1")
                    nc.vector.tensor_tensor(
                        out=h1[:], in0=psV[:], in1=vb1b_s[:],
                        op=mybir.AluOpType.add,
                    )
                    nc.vector.tensor_scalar_max(h1[:], h1[:], 0.0)
                    psT = ppool.tile([32, 128], FP32, tag="psmall")
                    nc.tensor.transpose(psT[:], h1[:], identf[:])
                    h1T = wpool.tile([32, 128], FP32, tag="h1T")
                    nc.vector.tensor_copy(h1T[:], psT[:])
                    psL = ppool.tile([128, 8], FP32, tag="psmall")
                    nc.tensor.matmul(
                        psL[:], lhsT=h1T[:], rhs=vw2_s[:], start=True, stop=True
                    )
                    j = 4 * t + b
                    nc.vector.tensor_tensor(
                        out=smxv[:, j * 8:(j + 1) * 8], in0=psL[:],
                        in1=vb2b_s[:], op=mybir.AluOpType.add,
                    )
            # batched softmax over groups of 8 along free dim
            nv = nvt * 4
            smx3 = smxv[:].rearrange("p (j c) -> p j c", c=8)
            vmax = wpool.tile([128, nv], FP32, tag="vmax")
            nc.vector.tensor_reduce(
                vmax[:], smx3, axis=mybir.AxisListType.X, op=mybir.AluOpType.max
            )
            nc.vector.tensor_tensor(
                out=smx3,
                in0=smx3,
                in1=vmax[:].rearrange("p (j c) -> p j c", c=1)
                .to_broadcast([128, nv, 8]),
                op=mybir.AluOpType.subtract,
            )
            nc.scalar.activation(smxv[:], smxv[:], mybir.ActivationFunctionType.Exp)
            vsum = wpool.tile([128, nv], FP32, tag="vsum")
            nc.vector.tensor_reduce(
                vsum[:], smx3, axis=mybir.AxisListType.X, op=mybir.AluOpType.add
            )
            nc.vector.reciprocal(vsum[:], vsum[:])
            nc.vector.tensor_tensor(
                out=smx3,
                in0=smx3,
                in1=vsum[:].rearrange("p (j c) -> p j c", c=1)
                .to_broadcast([128, nv, 8]),
                op=mybir.AluOpType.mult,
            )
            # node n = 128*j + p  ->  val_out row n
            nc.sync.dma_start(
                val_out.rearrange("(j p) c -> p j c", p=128),
                smxv[:].rearrange("p (j c) -> p j c", c=8),
            )

            # ---- phase C: edges, 4 groups x nslab slabs ----
            idxr_s = cpool.tile([128, EPALL // 16], I16, tag="idxr")
            nc.sync.dma_start(idxr_s[:], idxr16)
            idxc_s = cpool.tile([128, EPALL // 16], I16, tag="idxc")
            nc.sync.dma_start(idxc_s[:], idxc16)

            for grp in range(4):
                rh = grp >> 1
                ch = grp & 1
                tabr = tab[rh * HREF:(rh + 1) * HREF, :]
                tabc = tab[ch * HREF:(ch + 1) * HREF, :]
                for s in range(nslab):
                    tok0 = grp * EPG + s * TOK
                    isl = slice(tok0 // 16, (tok0 + TOK) // 16)
                    gA = wpool.tile([128, SW, 128], BF16, tag="gA")
                    nc.gpsimd.dma_gather(
                        gA[:], tabr, idxr_s[:, isl], TOK, TOK, 128,
                        single_packet=False,
                    )
                    gB = wpool.tile([128, SW, 128], BF16, tag="gB")
                    nc.gpsimd.dma_gather(
                        gB[:], tabc, idxc_s[:, isl], TOK, TOK, 128,
                        single_packet=False,
                    )
                    z = wpool.tile([128, SW * 32], BF16, tag="z")
                    nc.vector.tensor_tensor(
                        out=z[:].rearrange("p (j c) -> p j c", c=32),
                        in0=gA[:, :, 0:32],
                        in1=gB[:, :, 32:64],
                        op=mybir.AluOpType.add,
                    )
                    # transpose 4 edge-columns at a time; relu on psum->sbuf
                    zT = wpool.tile([128, SW * 32], BF16, tag="zT")
                    for q in range(0, nzt, 4):
                        qn = min(4, nzt - q)
                        psZ = ppool.tile([128, qn * 128], BF16, tag="psbig")
                        for b in range(qn):
                            nc.tensor.transpose(
                                psZ[:, b * 128:(b + 1) * 128],
                                z[:, (q + b) * 128:(q + b + 1) * 128],
                                ident[:],
                            )
                        nc.scalar.activation(
                            zT[:, q * 128:(q + qn) * 128],
                            psZ[:],
                            mybir.ActivationFunctionType.Relu,
                        )
                    # block-diag 32->4 matmul: logits for 512 edges per block
                    psM = ppool.tile([128, nzt * 16], FP32, tag="psbig")
                    for b in range(nzt):
                        nc.tensor.matmul(
                            psM[:, b * 16:(b + 1) * 16],
                            lhsT=zT[:, b * 128:(b + 1) * 128],
                            rhs=w2blk_s[:],
                            start=True,
                            stop=True,
                        )
                    logits = wpool.tile([128, SW * 4], FP32, tag="logits")
                    nc.vector.tensor_tensor(
                        out=logits[:],
                        in0=psM[:],
                        in1=bb2b_s[:].rearrange("p (j c) -> p j c", j=1)
                        .to_broadcast([128, nzt, 16]),
                        op=mybir.AluOpType.add,
                    )
                    # softmax over innermost 4
                    lg3 = logits[:].rearrange("p (j c) -> p j c", c=4)
                    bmax = wpool.tile([128, SW], FP32, tag="bmax")
                    nc.vector.tensor_reduce(
                        bmax[:], lg3, axis=mybir.AxisListType.X,
                        op=mybir.AluOpType.max,
                    )
                    nc.vector.tensor_tensor(
                        out=lg3, in0=lg3,
                        in1=bmax[:].rearrange("p (j c) -> p j c", c=1)
                        .to_broadcast([128, SW, 4]),
                        op=mybir.AluOpType.subtract,
                    )
                    nc.scalar.activation(
                        logits[:], logits[:], mybir.ActivationFunctionType.Exp
                    )
                    bsum = wpool.tile([128, SW], FP32, tag="bsum")
                    nc.vector.tensor_reduce(
                        bsum[:], lg3, axis=mybir.AxisListType.X,
                        op=mybir.AluOpType.add,
                    )
                    nc.vector.reciprocal(bsum[:], bsum[:])
                    bt = wpool.tile([128, SW * 4], FP32, tag="bt")
                    bt3 = bt[:].rearrange("p (j c) -> p j c", c=4)
                    nc.vector.tensor_tensor(
                        out=bt3, in0=lg3,
                        in1=bsum[:].rearrange("p (j c) -> p j c", c=1)
                        .to_broadcast([128, SW, 4]),
                        op=mybir.AluOpType.mult,
                    )
                    # token t = tok0 + j*128 + p  ->  bt_out row t
                    nc.sync.dma_start(
                        bt_out.rearrange("(j p) c -> p j c", p=128)
                        [:, tok0 // 128:tok0 // 128 + SW, :],
                        bt3,
                    )

    nc.compile()
    return nc


# ------------------------------------------------------------------
# host-side glue
# ------------------------------------------------------------------

def prepare_in_maps(P, node_features, edge_index, vw1, vb1, vw2, vb2,
                    bw1, bb1, bw2, bb2):
    """Build per-core input maps. Returns (in_maps, slot_maps) where
    slot_maps[c][t] = original local edge id of token t (or -1 for pads)."""
    NC = P["n_cores"]
    N = P["n_nodes"]
    NT = P["ntab"]
    HREF = NT // 2
    EPG = P["epg"]
    NSP = P["nsp"]
    NS = P["node_shard"]
    EC = P["e_per_core"]
    EPALL = 4 * EPG

    nf = np.asarray(node_features, np.float32)
    ei = np.asarray(edge_index)

    nfT = np.zeros((128, NT), ml_dtypes.bfloat16)
    nfT[:, :N] = nf.T.astype(ml_dtypes.bfloat16)

    w1ab = np.concatenate(
        [np.asarray(bw1[:128], np.float32), np.asarray(bw1[128:], np.float32)],
        axis=1,
    ).astype(ml_dtypes.bfloat16)  # [128, 64]
    biasAB = np.tile(
        np.concatenate([np.asarray(bb1, np.float32), np.zeros(32, np.float32)])[None],
        (128, 1),
    )
    vb1b = np.tile(np.asarray(vb1, np.float32)[None], (128, 1))
    vb2b = np.tile(np.asarray(vb2, np.float32)[None], (128, 1))
    w2blk = np.zeros((128, 16), np.float32)
    for m in range(4):
        w2blk[32 * m:32 * (m + 1), 4 * m:4 * (m + 1)] = np.asarray(bw2, np.float32)
    w2blk = w2blk.astype(ml_dtypes.bfloat16)
    bb2b = np.tile(np.tile(np.asarray(bb2, np.float32), 4)[None], (128, 1))

    shared = dict(
        nfT=nfT, w1ab=w1ab, biasAB=biasAB,
        vw1=np.asarray(vw1, np.float32), vb1b=vb1b,
        vw2=np.asarray(vw2, np.float32), vb2b=vb2b,
        w2blk=w2blk, bb2b=bb2b,
    )

    def wrap16(vals):
        # token i -> [i % 16, i // 16], replicated to 128 partitions
        out = np.zeros((16, EPALL // 16), np.int16)
        i = np.arange(EPALL)
        out[i % 16, i // 16] = vals
        return np.tile(out, (8, 1))

    in_maps = []
    slot_maps = []
    for c in range(NC):
        row = ei[0, c * EC:(c + 1) * EC].astype(np.int64)
        col = ei[1, c * EC:(c + 1) * EC].astype(np.int64)
        grp = (row >= HREF).astype(np.int64) * 2 + (col >= HREF).astype(np.int64)

        idxr = np.zeros(EPALL, np.int16)
        idxc = np.zeros(EPALL, np.int16)
        slots = np.full(EPALL, -1, np.int64)
        for g in range(4):
            ids = np.nonzero(grp == g)[0]
            assert len(ids) <= EPG, (
                f"edge group {g} overflow: {len(ids)} > {EPG}"
            )
            t0 = g * EPG
            slots[t0:t0 + len(ids)] = ids
            idxr[t0:t0 + len(ids)] = (row[ids] - (g >> 1) * HREF).astype(np.int16)
            idxc[t0:t0 + len(ids)] = (col[ids] - (g & 1) * HREF).astype(np.int16)

        ns0 = c * NS
        nfsT = np.zeros((128, NSP), np.float32)
        nfsT[:, :NS] = nf[ns0:ns0 + NS].T

        m = dict(shared)
        m.update(nfsT=nfsT, idxr16=wrap16(idxr), idxc16=wrap16(idxc))
        in_maps.append(m)
        slot_maps.append(slots)
    return in_maps, slot_maps


def postprocess(P, results, slot_maps, edge_row):
    NC = P["n_cores"]
    N = P["n_nodes"]
    NS = P["node_shard"]
    EC = P["e_per_core"]
    valences = np.concatenate(
        [results[c]["val_out"][:NS] for c in range(NC)], axis=0
    )[:N].astype(np.float32)
    bond_types = np.zeros((NC * EC, 4), np.float32)
    for c in range(NC):
        slots = slot_maps[c]
        valid = slots >= 0
        bond_types[c * EC + slots[valid]] = results[c]["bt_out"][valid]
    bond_order = bond_types @ np.asarray(BOND_W, np.float32)
    deg = np.bincount(edge_row, weights=bond_order, minlength=N)[:N]
    pv = (np.argmax(valences, axis=1) + 1).astype(np.float32)
    violation = np.float32(np.mean((deg.astype(np.float32) - pv) ** 2))
    return violation, valences, bond_types


_CACHED_NC = {}


def _get_nc(P):
    key = "v2"
    if key not in _CACHED_NC:
        _CACHED_NC[key] = build_nc(P)
    return _CACHED_NC[key]


_LAST_RESULTS = None


def kernel(node_features, edge_index, vw1, vb1, vw2, vb2, bw1, bb1, bw2, bb2):
    global _LAST_RESULTS
    from concourse import bass_utils

    P = default_params()
    nc = _get_nc(P)
    in_maps, slot_maps = prepare_in_maps(
        P, node_features, edge_index, vw1, vb1, vw2, vb2, bw1, bb1, bw2, bb2
    )
    trace = bool(os.environ.get("KERNEL_TRACE"))
    try:
        res = bass_utils.run_bass_kernel_spmd(
            nc, in_maps, core_ids=list(range(P["n_cores"])), trace=trace
        )
    except ModuleNotFoundError:
        # axon NTFF hook unavailable in this environment -> no trace
        res = bass_utils.run_bass_kernel_spmd(
            nc, in_maps, core_ids=list(range(P["n_cores"])), trace=False
        )
    _LAST_RESULTS = res
    edge_row = np.asarray(edge_index)[0].astype(np.int64)
    return postprocess(P, res.results, slot_maps, edge_row)
